# revision 65
# baseline (speedup 1.0000x reference)
"""3-layer GraphSAGE + classifier + log_softmax on 8 Trainium2 NeuronCores.

Self-contained: host-side sharding/packing + Bass/Tile device kernel.

Strategy
--------
concat([x, agg]) @ W  ==  x @ W_top + Ahat @ (x @ W_bot)   (linearity)
so aggregation happens in the 256-dim projected space.

- Nodes are permuted into 704 tiles of 128 (in-degree balanced), 88 tiles/core.
- Per layer: phase A computes r = x@W_top + b and p = x@W_bot per owned tile.
  p is written fp16 in two halves; each half is AllGathered into its own
  table (45056 rows) as soon as the half is computed, hiding collective
  latency under the remaining phase-A work.
- Phase B: per dst tile, gather p[src] rows for its in-edges via dma_gather
  (int16 indices; 4 overlapping windows, 2 per table). Window 0/2 carry a
  fixed 384 edges per tile (zero padding); windows 1/3 carry the remainder
  with trailing -1 indices that the gather ucode truncates per core.
  Gathers round-robin the 4 SWDGE queues so all 4 Q7 core pairs generate
  descriptors concurrently. The weighted one-hot selection matrices
  S[e, d] = wn_e * (dst_local_e == d) are precomputed on the host and
  DMAed; agg = sum_c S_c.T @ msg_c accumulates on the PE.
  x_next = relu(agg + r).
- x_next is transposed on the PE (2x 128x128) to feed the next layer's
  stationary operand; the classifier (768->7) runs per tile in layer-3
  phase B; the log_softmax runs batched (max/exp/sum per 4-tile block,
  single ln at the end).
"""

import numpy as np

import concourse.bass as bass
import concourse.mybir as mybir
import concourse.tile as tile
from concourse import bacc
from concourse.bass_utils import run_bass_kernel_spmd
from concourse.masks import make_identity

# problem constants
N = 89250
IN_F = 500
HID = 256
NCLS = 7
FPAD = 512  # padded input feature dim

NC = 8  # cores
P = 128
NT = 704  # node tiles
TPC = NT // NC  # 88 tiles per core
NPAD = NT * P  # 90112
NPC = TPC * P  # 11264 nodes per core
HT = TPC // 2  # 44 tiles per half
HROWS = HT * P  # 5632 rows per half per core
TAB = NC * HROWS  # 45056 rows per AllGathered table

WCAP = 32768  # int16 index reach
W1BASE = TAB - WCAP  # 12288: base row of windows 1/3 within their table
SPLIT0 = 384  # fixed edges per tile routed to window 0 (and window 2)
G4 = 4  # tiles per gather group
LA = 4  # lookahead groups for window-0/1 gathers (hide AllGather b)

f32 = mybir.dt.float32
f16 = mybir.dt.float16
i16 = mybir.dt.int16
i32 = mybir.dt.int32

_compile_cache = {}


# --------------------------------------------------------------------------
# host-side prep
# --------------------------------------------------------------------------

def _assign_tiles(in_deg):
    """LPT: assign node ids (0..NPAD) to (tile, slot), balancing in-edges."""
    import heapq

    order = np.argsort(-in_deg, kind="stable")
    heap = [(0, t) for t in range(NT)]
    heapq.heapify(heap)
    counts = np.zeros(NT, np.int32)
    newpos = np.empty(NPAD, np.int64)
    for v in order:
        load, t = heapq.heappop(heap)
        newpos[v] = t * P + counts[t]
        counts[t] += 1
        if counts[t] < P:
            heapq.heappush(heap, (load + int(in_deg[v]), t))
    return newpos


def _ru16(x):
    return (int(x) + 15) // 16 * 16


def prep(x, edge_index, edge_weight):
    src = edge_index[0].astype(np.int64)
    dst = edge_index[1].astype(np.int64)
    ew = edge_weight.astype(np.float32)

    cnt = np.bincount(dst, minlength=N).astype(np.float32)
    wn = ew / np.maximum(cnt[dst], 1.0)

    in_deg = np.zeros(NPAD, np.int64)
    in_deg[:N] = np.bincount(dst, minlength=N)
    newpos = _assign_tiles(in_deg)

    s2 = newpos[src]
    d2 = newpos[dst]
    # table row of each source: owner core c, local row jl; half A = first
    # 44 tiles of the core, half B = rest.  trow = c*HROWS + (jl mod HROWS)
    c_own = s2 // NPC
    jl = s2 % NPC
    is_b = jl >= HROWS
    trow = c_own * HROWS + (jl - HROWS * is_b)

    tile_of = d2 // P
    dl = (d2 % P).astype(np.int64)

    order = np.argsort(tile_of, kind="stable")
    trow_o, isb_o, dl_o, wn_o = trow[order], is_b[order], dl[order], wn[order]
    tile_o = tile_of[order]
    starts = np.searchsorted(tile_o, np.arange(NT + 1))

    # per (tile, window): sorted index lists + (dl, wn) in slot order
    # windows: 0 = A[0:32768), 1 = A[12288:45056), 2/3 same for B
    tw_idx = [[None] * 4 for _ in range(NT)]
    tw_dl = [[None] * 4 for _ in range(NT)]
    tw_wn = [[None] * 4 for _ in range(NT)]
    for t in range(NT):
        lo, hi = starts[t], starts[t + 1]
        tr, ib = trow_o[lo:hi], isb_o[lo:hi]
        dd, ww = dl_o[lo:hi], wn_o[lo:hi]
        for half in range(2):
            sel = np.nonzero(ib == half)[0]
            o = sel[np.argsort(tr[sel], kind="stable")]
            n = len(o)
            assert n >= SPLIT0, f"tile {t} half {half}: only {n} edges"
            assert tr[o[SPLIT0 - 1]] < WCAP, f"tile {t}: w0 split infeasible"
            assert tr[o[SPLIT0]] >= W1BASE, f"tile {t}: w1 split infeasible"
            w0, w1 = 2 * half, 2 * half + 1
            tw_idx[t][w0] = tr[o[:SPLIT0]].astype(np.int16)
            tw_idx[t][w1] = (tr[o[SPLIT0:]] - W1BASE).astype(np.int16)
            tw_dl[t][w0], tw_dl[t][w1] = dd[o[:SPLIT0]], dd[o[SPLIT0:]]
            tw_wn[t][w0], tw_wn[t][w1] = ww[o[:SPLIT0]], ww[o[SPLIT0:]]

    # per tile-slot chunk budgets (uniform across cores for SPMD); tiles are
    # chunk-aligned inside group gathers, so pad each (tile, window) to a
    # multiple of 128 with repeats of the last index (row-buffer-hit reads)
    cb = np.zeros((TPC, 4), np.int64)
    for tl in range(TPC):
        for w in range(4):
            mx = max(len(tw_idx[c * TPC + tl][w]) for c in range(NC))
            cb[tl, w] = (mx + P - 1) // P
    kc = cb.sum(axis=1)  # chunks per tile
    soff = np.zeros(TPC + 1, np.int64)
    soff[1:] = np.cumsum(kc)
    sct = int(soff[-1])

    # group-of-G4 gather packing: per (group, window) one gather whose index
    # list is the concat of the group's tiles (each padded to cb*128)
    ng4 = TPC // G4
    cb4 = np.zeros((ng4, 4), np.int64)  # chunks per (group, window)
    for g in range(ng4):
        cb4[g] = cb[g * G4:(g + 1) * G4].sum(axis=0)
    go4 = np.zeros((ng4, 5), np.int64)  # gidx column offsets (int16 cols)
    for g in range(ng4):
        go4[g, 1:] = np.cumsum(cb4[g] * 8)
    gmax = int(go4[:, 4].max())

    # S chunk numbering: per tile, w1 chunks then w3 chunks (DVE-built);
    # w0/w2 chunks (3 each, fixed) are host-precomputed dense and DMAed
    kv = (cb[:, 1] + cb[:, 3]).astype(np.int64)  # DVE-built chunks per tile
    voff = np.zeros(TPC + 1, np.int64)
    voff[1:] = np.cumsum(kv)
    vct = int(voff[-1])

    gidx = np.zeros((NC, ng4, P, gmax), np.int16)
    mdl = np.zeros((NC, P, vct), np.float16)
    mwn = np.zeros((NC, P, vct), np.float16)
    sdh = np.zeros((NC, P, TPC * 6, P), np.float16)
    for t in range(NT):
        c, tl = t // TPC, t % TPC
        g, ti = tl // G4, tl % G4
        cv0 = 0
        for w in range(4):
            idx = tw_idx[t][w]
            n = len(idx)
            b = int(cb[tl, w]) * P
            arr = np.full(b, idx[-1], np.int16)
            arr[:n] = idx
            wrapped = arr.reshape(-1, 16).T  # [16, b/16]
            coff = int(go4[g, w]) + int(cb[g * G4:tl, w].sum()) * 8
            gidx[c, g, :, coff:coff + b // 16] = np.tile(wrapped, (8, 1))
            sl = np.arange(n)
            if w % 2 == 0:  # dense host S: w0 -> chunks 0-2, w2 -> 3-5
                h0 = tl * 6 + (0 if w == 0 else 3)
                sdh[c, sl % P, h0 + sl // P,
                    tw_dl[t][w].astype(np.int64)] = tw_wn[t][w]
            else:
                mdl[c, sl % P, voff[tl] + cv0 + sl // P] = tw_dl[t][w]
                mwn[c, sl % P, voff[tl] + cv0 + sl // P] = tw_wn[t][w]
                cv0 += int(cb[tl, w])

    # transposed, padded, permuted node features
    xT = np.zeros((FPAD, NPAD), np.float16)
    xT[:IN_F, newpos[:N]] = x.T

    return {
        "newpos": newpos,
        "cb": cb,
        "cb4": cb4,
        "go4": go4,
        "kc": kc,
        "kv": kv,
        "voff": voff,
        "vct": vct,
        "gmax": gmax,
        "xT": xT,
        "mdl": mdl,
        "mwn": mwn,
        "sdh": sdh,
        "gidx": gidx,
    }


def pack_weights(W1, b1, W2, b2, W3, b3, Wl, bl):
    def chunk_rhs(W, kchunks, dtype):
        # [F, 512] -> [128, kchunks, 512]
        F = W.shape[0]
        Wp = np.zeros((kchunks * P, 512), np.float32)
        Wp[:F] = W
        return np.ascontiguousarray(
            Wp.reshape(kchunks, P, 512).transpose(1, 0, 2)
        ).astype(dtype)

    w1cat = np.concatenate([W1[:IN_F], W1[IN_F:]], axis=1)  # [500, 512]
    w2cat = np.concatenate([W2[:HID], W2[HID:]], axis=1)  # [256, 512]
    w3cat = np.concatenate([W3[:HID], W3[HID:]], axis=1)
    wl = np.ascontiguousarray(
        Wl.reshape(6, P, NCLS).transpose(1, 0, 2)
    ).astype(np.float16)  # [128, 6, 7]
    return {
        "w1": chunk_rhs(w1cat, 4, np.float16),
        "w2": chunk_rhs(w2cat, 2, np.float16),
        "w3": chunk_rhs(w3cat, 2, np.float16),
        "wl": wl,
        "b1": np.tile(b1[None, :], (P, 1)).astype(np.float32),
        "b2": np.tile(b2[None, :], (P, 1)).astype(np.float32),
        "b3": np.tile(b3[None, :], (P, 1)).astype(np.float32),
        "bl": np.tile(bl[None, :], (P, 1)).astype(np.float32),
    }


# --------------------------------------------------------------------------
# device kernel
# --------------------------------------------------------------------------

def build(struct):
    cb = struct["cb"]
    cb4 = struct["cb4"]
    go4 = struct["go4"]
    kc = struct["kc"]
    kv = struct["kv"]
    voff = struct["voff"]
    vct = struct["vct"]
    gmax = struct["gmax"]
    ng4 = TPC // G4
    cb4max = [int(cb4[:, w].max()) for w in range(4)]

    nc = bacc.Bacc(
        "TRN2", target_bir_lowering=False, debug=False, num_devices=NC,
        num_swdge_queues=4,
    )

    xTc = nc.dram_tensor("xTc", [FPAD, NPC], f16, kind="ExternalInput")
    mdl_d = nc.dram_tensor("mdl", [P, vct], f16, kind="ExternalInput")
    mwn_d = nc.dram_tensor("mwn", [P, vct], f16, kind="ExternalInput")
    sdh_d = nc.dram_tensor("sdh", [P, TPC * 6, P], f16, kind="ExternalInput")
    gidx_d = nc.dram_tensor("gidx", [ng4, P, gmax], i16, kind="ExternalInput")
    w1_d = nc.dram_tensor("w1", [P, 4, 512], f16, kind="ExternalInput")
    w2_d = nc.dram_tensor("w2", [P, 2, 512], f16, kind="ExternalInput")
    w3_d = nc.dram_tensor("w3", [P, 2, 512], f16, kind="ExternalInput")
    wl_d = nc.dram_tensor("wl", [P, 6, NCLS], f16, kind="ExternalInput")
    b1_d = nc.dram_tensor("b1", [P, HID], f32, kind="ExternalInput")
    b2_d = nc.dram_tensor("b2", [P, HID], f32, kind="ExternalInput")
    b3_d = nc.dram_tensor("b3", [P, HID], f32, kind="ExternalInput")
    bl_d = nc.dram_tensor("bl", [P, NCLS], f32, kind="ExternalInput")
    out_d = nc.dram_tensor("out", [NPC, NCLS], f32, kind="ExternalOutput")

    with tile.TileContext(nc) as tc:
        with (
            tc.tile_pool(name="dram", bufs=1, space="DRAM") as dram,
            tc.tile_pool(name="const", bufs=1) as cpool,
            tc.tile_pool(name="lx", bufs=4) as lxpool,
            tc.tile_pool(name="stage", bufs=3) as stpool,
            tc.tile_pool(name="msga", bufs=LA + 1) as msgapool,
            tc.tile_pool(name="msgb", bufs=3) as msgbpool,
            tc.tile_pool(name="sbuild", bufs=3) as sbpool,
            tc.tile_pool(name="psa", bufs=2, space="PSUM") as psa,
            tc.tile_pool(name="psagg", bufs=3, space="PSUM") as psagg,
            tc.tile_pool(name="pstr", bufs=2, space="PSUM") as pstr,
            tc.tile_pool(name="pscls", bufs=1, space="PSUM") as pscls,
        ):
            # ---- DRAM intermediates
            p_loc = [
                [dram.tile([HROWS, HID], f16, name=f"p{i}loc{h}")
                 for h in range(2)]
                for i in range(3)
            ]
            p_full = [
                [dram.tile([TAB, HID], f16, addr_space="Shared",
                           name=f"p{i}full{h}") for h in range(2)]
                for i in range(3)
            ]
            r_dram = [dram.tile([NPC, HID], f16, name=f"r{i}d") for i in range(3)]

            # ---- constants
            w1_sb = cpool.tile([P, 4, 512], f16)
            nc.sync.dma_start(w1_sb[:], w1_d[:])
            w2_sb = cpool.tile([P, 2, 512], f16)
            nc.sync.dma_start(w2_sb[:], w2_d[:])
            w3_sb = cpool.tile([P, 2, 512], f16)
            nc.sync.dma_start(w3_sb[:], w3_d[:])
            wl_sb = cpool.tile([P, 6, NCLS], f16)
            nc.sync.dma_start(wl_sb[:], wl_d[:])
            b_sb = []
            for name, t in (("b1", b1_d), ("b2", b2_d), ("b3", b3_d)):
                bt = cpool.tile([P, HID], f32, name=name + "sb")
                nc.sync.dma_start(bt[:], t[:])
                b_sb.append(bt)
            bl_sb = cpool.tile([P, NCLS], f32)
            nc.sync.dma_start(bl_sb[:], bl_d[:])

            gidx_sb = cpool.tile([P, ng4, gmax], i16)
            nc.sync.dma_start(
                gidx_sb[:],
                gidx_d[:].rearrange("g p c -> p g c"),
            )

            ident = cpool.tile([P, P], f16)
            make_identity(nc, ident[:])

            mdl_sb = cpool.tile([P, vct], f16)
            nc.sync.dma_start(mdl_sb[:], mdl_d[:])
            mwn_sb = cpool.tile([P, vct], f16)
            nc.sync.dma_start(mwn_sb[:], mwn_d[:])
            iota_i = cpool.tile([P, P], i32)
            nc.gpsimd.iota(iota_i[:], pattern=[[1, P]], base=0,
                           channel_multiplier=0)
            iota_f = cpool.tile([P, P], f16)
            nc.vector.tensor_copy(iota_f[:], iota_i[:])

            lgall = cpool.tile([P, TPC, NCLS], f32)
            sm_all = cpool.tile([P, TPC], f32)

            # ---- layer-1 phase A (by pairs of tiles): r = x@Wtop+b, p = x@Wbot
            def pair_rows(buf, pr):
                return buf[pr * 2 * P:(pr + 1) * 2 * P, :].rearrange(
                    "(t p) c -> p t c", p=P
                )

            def phase_a_pair(pr):
                half, lpr = pr // (HT // 2), pr % (HT // 2)
                lx = lxpool.tile([P, 4, 2 * P], f16, name="lx", tag="lx")
                nc.sync.dma_start(
                    lx[:],
                    xTc[:, pr * 2 * P:(pr + 1) * 2 * P].rearrange(
                        "(k p) c -> p k c", p=P
                    ),
                )
                rst = stpool.tile([P, 2, HID], f16, name="rst", tag="rst")
                pst = stpool.tile([P, 2, HID], f16, name="pst", tag="pst")
                for gt in range(2):
                    ps = psa.tile([P, 512], f32, name="psA", tag="psA")
                    for k in range(4):
                        nc.tensor.matmul(
                            out=ps[:], lhsT=lx[:, k, gt * P:(gt + 1) * P],
                            rhs=w1_sb[:, k, :],
                            start=(k == 0), stop=(k == 3),
                        )
                    nc.vector.tensor_tensor(
                        out=rst[:, gt, :], in0=ps[:, :HID], in1=b_sb[0][:],
                        op=mybir.AluOpType.add,
                    )
                    nc.vector.tensor_copy(pst[:, gt, :], ps[:, HID:])
                nc.sync.dma_start(pair_rows(r_dram[0], pr), rst[:])
                nc.sync.dma_start(pair_rows(p_loc[0][half], lpr), pst[:])

            def fire_ag(layer, half):
                nc.gpsimd.collective_compute(
                    "AllGather",
                    mybir.AluOpType.bypass,
                    replica_groups=[list(range(NC))],
                    ins=[p_loc[layer][half].opt()],
                    outs=[p_full[layer][half].opt()],
                )

            # ---- phase B by groups of G4 tiles
            def issue_gathers(layer, g, ws):
                ms = {}
                for w in ws:
                    pool = msgapool if w < 2 else msgbpool
                    m = pool.tile(
                        [P, cb4max[w], HID], f16, name=f"m{w}", tag=f"m{w}"
                    )
                    cols = int(cb4[g, w])
                    ni = cols * P
                    nc.gpsimd.dma_gather(
                        out_ap=m[:, :cols, :],
                        in_ap=p_full[layer][w // 2][
                            (W1BASE if w % 2 else 0):, :
                        ],
                        idxs_ap=gidx_sb[:, g, int(go4[g, w]):int(go4[g, w + 1])],
                        num_idxs=ni,
                        num_idxs_reg=ni,
                        elem_size=HID,
                        single_packet=(ni <= 1024),
                        queue_num=w,
                    )
                    ms[w] = m
                return ms

            def phase_b_group(layer, g, msgs):
                last = layer == 2
                t0 = g * G4
                kvg = int(voff[t0 + G4] - voff[t0])
                o0 = int(voff[t0])
                s_v = sbpool.tile([P, kvg, P], f16, name="sv", tag="sv")
                nc.vector.tensor_tensor(
                    out=s_v[:],
                    in0=mdl_sb[:, o0:o0 + kvg].unsqueeze(2).broadcast_to(
                        [P, kvg, P]
                    ),
                    in1=iota_f[:].unsqueeze(1).broadcast_to([P, kvg, P]),
                    op=mybir.AluOpType.is_equal,
                )
                nc.vector.tensor_tensor(
                    out=s_v[:],
                    in0=s_v[:],
                    in1=mwn_sb[:, o0:o0 + kvg].unsqueeze(2).broadcast_to(
                        [P, kvg, P]
                    ),
                    op=mybir.AluOpType.mult,
                )
                s_h = sbpool.tile([P, G4 * 6, P], f16, name="sh", tag="sh")
                nc.sync.dma_start(
                    s_h[:], sdh_d[:, t0 * 6:(t0 + G4) * 6, :]
                )
                rst = stpool.tile([P, G4, HID], f16, name="rl", tag="rl")
                nc.sync.dma_start(
                    rst[:],
                    r_dram[layer][t0 * P:(t0 + G4) * P, :].rearrange(
                        "(t p) c -> p t c", p=P
                    ),
                )
                aggs = []
                for ti in range(G4):
                    tl = t0 + ti
                    agg = psagg.tile([P, HID], f32, name="agg", tag="agg")
                    nchunks = int(kc[tl])
                    cv = int(voff[tl]) - o0
                    done = 0
                    for w in range(4):
                        cbo = int(cb[t0:tl, w].sum())
                        for j in range(int(cb[tl, w])):
                            if w % 2 == 0:
                                lhs = s_h[:, ti * 6 + (0 if w == 0 else 3) + j, :]
                            else:
                                lhs = s_v[:, cv, :]
                                cv += 1
                            nc.tensor.matmul(
                                out=agg[:],
                                lhsT=lhs,
                                rhs=msgs[w][:, cbo + j, :],
                                start=(done == 0),
                                stop=(done == nchunks - 1),
                            )
                            done += 1
                    aggs.append(agg)
                for ti in range(G4):
                    tl = t0 + ti
                    agg = aggs[ti]
                    xsum = stpool.tile([P, HID], f16, name="xsum", tag="xsum")
                    nc.vector.tensor_tensor(
                        out=xsum[:], in0=agg[:], in1=rst[:, ti, :],
                        op=mybir.AluOpType.add,
                    )
                    xn = stpool.tile([P, HID], f16, name="xn", tag="xn")
                    nc.scalar.activation(
                        xn[:], xsum[:], mybir.ActivationFunctionType.Relu
                    )
                    xts = []
                    for h in range(2):
                        tp = pstr.tile([P, P], f16, name="tp", tag="tp")
                        nc.tensor.transpose(
                            out=tp[:], in_=xn[:, h * P:(h + 1) * P],
                            identity=ident[:],
                        )
                        xt = stpool.tile([P, P], f16, name=f"xt{h}",
                                         tag=f"xt{h}")
                        nc.vector.tensor_copy(xt[:], tp[:])
                        xts.append(xt)
                    # fused next-layer phase A: r/p for layer+1 from x in SBUF
                    if not last:
                        half, lt = tl // HT, tl % HT
                        wsb = w2_sb if layer == 0 else w3_sb
                        ps = psa.tile([P, 512], f32, name="psA", tag="psA")
                        for k in range(2):
                            nc.tensor.matmul(
                                out=ps[:], lhsT=xts[k][:], rhs=wsb[:, k, :],
                                start=(k == 0), stop=(k == 1),
                            )
                        rs2 = stpool.tile([P, HID], f16, name="rs2", tag="rs2")
                        nc.vector.tensor_tensor(
                            out=rs2[:], in0=ps[:, :HID], in1=b_sb[layer + 1][:],
                            op=mybir.AluOpType.add,
                        )
                        ps2 = stpool.tile([P, HID], f16, name="ps2", tag="ps2")
                        nc.vector.tensor_copy(ps2[:], ps[:, HID:])
                        nc.sync.dma_start(
                            r_dram[layer + 1][tl * P:(tl + 1) * P, :].rearrange(
                                "(o p) c -> p (o c)", p=P
                            ),
                            rs2[:],
                        )
                        nc.sync.dma_start(
                            p_loc[layer + 1][half][
                                lt * P:(lt + 1) * P, :
                            ].rearrange("(o p) c -> p (o c)", p=P),
                            ps2[:],
                        )
                    # incremental classifier: lgall += x_l^T @ Wl[2l:2l+2]
                    cls = pscls.tile([P, NCLS], f32, name="cls", tag="cls")
                    for kk in range(2):
                        nc.tensor.matmul(
                            out=cls[:], lhsT=xts[kk][:],
                            rhs=wl_sb[:, 2 * layer + kk, :],
                            start=(kk == 0), stop=(kk == 1),
                        )
                    if layer == 0:
                        nc.vector.tensor_copy(lgall[:, tl, :], cls[:])
                    else:
                        nc.vector.tensor_tensor(
                            out=lgall[:, tl, :], in0=lgall[:, tl, :],
                            in1=cls[:], op=mybir.AluOpType.add,
                        )
                    if last:
                        nc.vector.tensor_tensor(
                            out=lgall[:, tl, :], in0=lgall[:, tl, :],
                            in1=bl_sb[:], op=mybir.AluOpType.add,
                        )

            # batched shifted-exp over a block of tiles (layer 3)
            def softmax_block(t0, nt):
                mx = stpool.tile([P, nt], f32, name="mx", tag="mx")
                nc.vector.tensor_reduce(
                    out=mx[:], in_=lgall[:, t0:t0 + nt, :],
                    axis=mybir.AxisListType.X, op=mybir.AluOpType.max,
                )
                nc.vector.tensor_tensor(
                    out=lgall[:, t0:t0 + nt, :], in0=lgall[:, t0:t0 + nt, :],
                    in1=mx[:].unsqueeze(2).broadcast_to([P, nt, NCLS]),
                    op=mybir.AluOpType.subtract,
                )
                ex = stpool.tile([P, nt, NCLS], f32, name="ex", tag="ex")
                nc.scalar.activation(
                    ex[:], lgall[:, t0:t0 + nt, :],
                    mybir.ActivationFunctionType.Exp,
                )
                nc.vector.tensor_reduce(
                    out=sm_all[:, t0:t0 + nt], in_=ex[:],
                    axis=mybir.AxisListType.X, op=mybir.AluOpType.add,
                )

            # ---- drive: layer-1 phase A, then 3 fused phase-B passes
            for pr in range(HT):
                phase_a_pair(pr)
                if pr == HT // 2 - 1:
                    fire_ag(0, 0)
            fire_ag(0, 1)
            for layer in range(3):
                # lookahead window-0/1 gathers overlap AllGather b's flight
                amsgs = [issue_gathers(layer, g, (0, 1)) for g in range(LA)]
                for g in range(ng4):
                    msgs = issue_gathers(layer, g, (2, 3))
                    if g + LA < ng4:
                        amsgs.append(issue_gathers(layer, g + LA, (0, 1)))
                    msgs.update(amsgs[g])
                    phase_b_group(layer, g, msgs)
                    if layer < 2:
                        if g == HT // G4 - 1:
                            fire_ag(layer + 1, 0)
                        elif g == ng4 - 1:
                            fire_ag(layer + 1, 1)
                    else:
                        softmax_block(g * G4, G4)

            lsm = cpool.tile([P, TPC], f32)
            nc.scalar.activation(
                lsm[:], sm_all[:], mybir.ActivationFunctionType.Ln
            )
            nc.vector.tensor_tensor(
                out=lgall[:], in0=lgall[:],
                in1=lsm[:].unsqueeze(2).broadcast_to([P, TPC, NCLS]),
                op=mybir.AluOpType.subtract,
            )
            nc.sync.dma_start(
                out_d[:].rearrange("(t p) j -> p t j", p=P), lgall[:]
            )

    nc.compile()
    return nc


# --------------------------------------------------------------------------
# entry point
# --------------------------------------------------------------------------

def kernel(x, edge_index, edge_weight, W1, b1, W2, b2, W3, b3, Wl, bl):
    x = np.asarray(x, dtype=np.float32)
    edge_index = np.asarray(edge_index)
    edge_weight = np.asarray(edge_weight, dtype=np.float32)

    pp = prep(x, edge_index, edge_weight)
    wts = pack_weights(
        np.asarray(W1, np.float32), np.asarray(b1, np.float32),
        np.asarray(W2, np.float32), np.asarray(b2, np.float32),
        np.asarray(W3, np.float32), np.asarray(b3, np.float32),
        np.asarray(Wl, np.float32), np.asarray(bl, np.float32),
    )

    key = (pp["vct"], pp["gmax"], tuple(pp["cb"].reshape(-1).tolist()))
    if key not in _compile_cache:
        _compile_cache[key] = build(pp)
    nc = _compile_cache[key]

    in_maps = []
    for c in range(NC):
        in_maps.append({
            "xTc": np.ascontiguousarray(pp["xT"][:, c * NPC:(c + 1) * NPC]),
            "mdl": np.ascontiguousarray(pp["mdl"][c]),
            "mwn": np.ascontiguousarray(pp["mwn"][c]),
            "sdh": np.ascontiguousarray(pp["sdh"][c]),
            "gidx": np.ascontiguousarray(pp["gidx"][c]),
            **wts,
        })

    res = run_bass_kernel_spmd(nc, in_maps, list(range(NC)))
    out_full = np.concatenate([res.results[c]["out"] for c in range(NC)], axis=0)
    return out_full[pp["newpos"][:N]].astype(np.float32)


if __name__ == "__main__":
    import time

    rng = np.random.default_rng(0)
    E = 899756
    ei = rng.integers(0, N, (2, E)).astype(np.int32)
    ew = rng.random(E, dtype=np.float32)
    x = rng.standard_normal((N, IN_F), dtype=np.float32)
    t0 = time.time()
    pp = prep(x, ei, ew)
    print("prep", time.time() - t0, "sct =", pp["sct"], "gmax =", pp["gmax"])


# revision 67
# speedup vs baseline: 1.0234x; 1.0234x over previous
"""3-layer GraphSAGE + classifier + log_softmax on 8 Trainium2 NeuronCores.

Self-contained: host-side sharding/packing + Bass/Tile device kernel.

Strategy
--------
concat([x, agg]) @ W  ==  x @ W_top + Ahat @ (x @ W_bot)   (linearity)
so aggregation happens in the 256-dim projected space.

- Nodes are permuted into 704 tiles of 128 (in-degree balanced), 88 tiles/core.
- Per layer: phase A computes r = x@W_top + b and p = x@W_bot per owned tile.
  p is written fp16 in two halves; each half is AllGathered into its own
  table (45056 rows) as soon as the half is computed, hiding collective
  latency under the remaining phase-A work.
- Phase B: per dst tile, gather p[src] rows for its in-edges via dma_gather
  (int16 indices; 4 overlapping windows, 2 per table). Window 0/2 carry a
  fixed 384 edges per tile (zero padding); windows 1/3 carry the remainder
  with trailing -1 indices that the gather ucode truncates per core.
  Gathers round-robin the 4 SWDGE queues so all 4 Q7 core pairs generate
  descriptors concurrently. The weighted one-hot selection matrices
  S[e, d] = wn_e * (dst_local_e == d) are precomputed on the host and
  DMAed; agg = sum_c S_c.T @ msg_c accumulates on the PE.
  x_next = relu(agg + r).
- x_next is transposed on the PE (2x 128x128) to feed the next layer's
  stationary operand; the classifier (768->7) runs per tile in layer-3
  phase B; the log_softmax runs batched (max/exp/sum per 4-tile block,
  single ln at the end).
"""

import numpy as np

import concourse.bass as bass
import concourse.mybir as mybir
import concourse.tile as tile
from concourse import bacc
from concourse.bass_utils import run_bass_kernel_spmd
from concourse.masks import make_identity

# problem constants
N = 89250
IN_F = 500
HID = 256
NCLS = 7
FPAD = 512  # padded input feature dim

NC = 8  # cores
P = 128
NT = 704  # node tiles
TPC = NT // NC  # 88 tiles per core
NPAD = NT * P  # 90112
NPC = TPC * P  # 11264 nodes per core
HT = TPC // 2  # 44 tiles per half
HROWS = HT * P  # 5632 rows per half per core
TAB = NC * HROWS  # 45056 rows per AllGathered table

WCAP = 32768  # int16 index reach
W1BASE = TAB - WCAP  # 12288: base row of windows 1/3 within their table
SPLIT0 = 384  # fixed edges per tile routed to window 0 (and window 2)
G4 = 4  # tiles per gather group
LA = 4  # lookahead groups for window-0/1 gathers (hide AllGather b)

f32 = mybir.dt.float32
f16 = mybir.dt.float16
i16 = mybir.dt.int16
i32 = mybir.dt.int32

_compile_cache = {}


# --------------------------------------------------------------------------
# host-side prep
# --------------------------------------------------------------------------

def _assign_tiles(in_deg):
    """LPT: assign node ids (0..NPAD) to (tile, slot), balancing in-edges."""
    import heapq

    order = np.argsort(-in_deg, kind="stable")
    heap = [(0, t) for t in range(NT)]
    heapq.heapify(heap)
    counts = np.zeros(NT, np.int32)
    newpos = np.empty(NPAD, np.int64)
    for v in order:
        load, t = heapq.heappop(heap)
        newpos[v] = t * P + counts[t]
        counts[t] += 1
        if counts[t] < P:
            heapq.heappush(heap, (load + int(in_deg[v]), t))
    return newpos


def _ru16(x):
    return (int(x) + 15) // 16 * 16


def prep(x, edge_index, edge_weight):
    src = edge_index[0].astype(np.int64)
    dst = edge_index[1].astype(np.int64)
    ew = edge_weight.astype(np.float32)

    cnt = np.bincount(dst, minlength=N).astype(np.float32)
    wn = ew / np.maximum(cnt[dst], 1.0)

    in_deg = np.zeros(NPAD, np.int64)
    in_deg[:N] = np.bincount(dst, minlength=N)
    newpos = _assign_tiles(in_deg)

    s2 = newpos[src]
    d2 = newpos[dst]
    # table row of each source: owner core c, local row jl; half A = first
    # 44 tiles of the core, half B = rest.  trow = c*HROWS + (jl mod HROWS)
    c_own = s2 // NPC
    jl = s2 % NPC
    is_b = jl >= HROWS
    trow = c_own * HROWS + (jl - HROWS * is_b)

    tile_of = d2 // P
    dl = (d2 % P).astype(np.int64)

    order = np.argsort(tile_of, kind="stable")
    trow_o, isb_o, dl_o, wn_o = trow[order], is_b[order], dl[order], wn[order]
    tile_o = tile_of[order]
    starts = np.searchsorted(tile_o, np.arange(NT + 1))

    # per (tile, window): sorted index lists + (dl, wn) in slot order
    # windows: 0 = A[0:32768), 1 = A[12288:45056), 2/3 same for B
    tw_idx = [[None] * 4 for _ in range(NT)]
    tw_dl = [[None] * 4 for _ in range(NT)]
    tw_wn = [[None] * 4 for _ in range(NT)]
    for t in range(NT):
        lo, hi = starts[t], starts[t + 1]
        tr, ib = trow_o[lo:hi], isb_o[lo:hi]
        dd, ww = dl_o[lo:hi], wn_o[lo:hi]
        for half in range(2):
            sel = np.nonzero(ib == half)[0]
            o = sel[np.argsort(tr[sel], kind="stable")]
            n = len(o)
            assert n >= SPLIT0, f"tile {t} half {half}: only {n} edges"
            assert tr[o[SPLIT0 - 1]] < WCAP, f"tile {t}: w0 split infeasible"
            assert tr[o[SPLIT0]] >= W1BASE, f"tile {t}: w1 split infeasible"
            w0, w1 = 2 * half, 2 * half + 1
            tw_idx[t][w0] = tr[o[:SPLIT0]].astype(np.int16)
            tw_idx[t][w1] = (tr[o[SPLIT0:]] - W1BASE).astype(np.int16)
            tw_dl[t][w0], tw_dl[t][w1] = dd[o[:SPLIT0]], dd[o[SPLIT0:]]
            tw_wn[t][w0], tw_wn[t][w1] = ww[o[:SPLIT0]], ww[o[SPLIT0:]]

    # per tile-slot chunk budgets (uniform across cores for SPMD); tiles are
    # chunk-aligned inside group gathers, so pad each (tile, window) to a
    # multiple of 128 with repeats of the last index (row-buffer-hit reads)
    cb = np.zeros((TPC, 4), np.int64)
    for tl in range(TPC):
        for w in range(4):
            mx = max(len(tw_idx[c * TPC + tl][w]) for c in range(NC))
            cb[tl, w] = (mx + P - 1) // P
    kc = cb.sum(axis=1)  # chunks per tile
    soff = np.zeros(TPC + 1, np.int64)
    soff[1:] = np.cumsum(kc)
    sct = int(soff[-1])

    # group-of-G4 gather packing: per (group, window) one gather whose index
    # list is the concat of the group's tiles (each padded to cb*128)
    ng4 = TPC // G4
    cb4 = np.zeros((ng4, 4), np.int64)  # chunks per (group, window)
    for g in range(ng4):
        cb4[g] = cb[g * G4:(g + 1) * G4].sum(axis=0)
    go4 = np.zeros((ng4, 5), np.int64)  # gidx column offsets (int16 cols)
    for g in range(ng4):
        go4[g, 1:] = np.cumsum(cb4[g] * 8)
    gmax = int(go4[:, 4].max())

    # S chunk numbering: per tile, w1 chunks then w3 chunks (DVE-built);
    # w0/w2 chunks (3 each, fixed) are host-precomputed dense and DMAed
    kv = (cb[:, 1] + cb[:, 3]).astype(np.int64)  # DVE-built chunks per tile
    voff = np.zeros(TPC + 1, np.int64)
    voff[1:] = np.cumsum(kv)
    vct = int(voff[-1])

    gidx = np.zeros((NC, ng4, P, gmax), np.int16)
    mdl = np.zeros((NC, P, vct), np.float16)
    mwn = np.zeros((NC, P, vct), np.float16)
    sdh = np.zeros((NC, P, TPC * 6, P), np.float16)
    for t in range(NT):
        c, tl = t // TPC, t % TPC
        g, ti = tl // G4, tl % G4
        cv0 = 0
        for w in range(4):
            idx = tw_idx[t][w]
            n = len(idx)
            b = int(cb[tl, w]) * P
            arr = np.full(b, idx[-1], np.int16)
            arr[:n] = idx
            wrapped = arr.reshape(-1, 16).T  # [16, b/16]
            coff = int(go4[g, w]) + int(cb[g * G4:tl, w].sum()) * 8
            gidx[c, g, :, coff:coff + b // 16] = np.tile(wrapped, (8, 1))
            sl = np.arange(n)
            if w % 2 == 0:  # dense host S: w0 -> chunks 0-2, w2 -> 3-5
                h0 = tl * 6 + (0 if w == 0 else 3)
                sdh[c, sl % P, h0 + sl // P,
                    tw_dl[t][w].astype(np.int64)] = tw_wn[t][w]
            else:
                mdl[c, sl % P, voff[tl] + cv0 + sl // P] = tw_dl[t][w]
                mwn[c, sl % P, voff[tl] + cv0 + sl // P] = tw_wn[t][w]
                cv0 += int(cb[tl, w])

    # transposed, padded, permuted node features
    xT = np.zeros((FPAD, NPAD), np.float16)
    xT[:IN_F, newpos[:N]] = x.T

    return {
        "newpos": newpos,
        "cb": cb,
        "cb4": cb4,
        "go4": go4,
        "kc": kc,
        "kv": kv,
        "voff": voff,
        "vct": vct,
        "gmax": gmax,
        "xT": xT,
        "mdl": mdl,
        "mwn": mwn,
        "sdh": sdh,
        "gidx": gidx,
    }


def pack_weights(W1, b1, W2, b2, W3, b3, Wl, bl):
    def chunk_rhs(W, kchunks, dtype):
        # [F, 512] -> [128, kchunks, 512]
        F = W.shape[0]
        Wp = np.zeros((kchunks * P, 512), np.float32)
        Wp[:F] = W
        return np.ascontiguousarray(
            Wp.reshape(kchunks, P, 512).transpose(1, 0, 2)
        ).astype(dtype)

    w1cat = np.concatenate([W1[:IN_F], W1[IN_F:]], axis=1)  # [500, 512]
    w2cat = np.concatenate([W2[:HID], W2[HID:]], axis=1)  # [256, 512]
    w3cat = np.concatenate([W3[:HID], W3[HID:]], axis=1)
    wl = np.ascontiguousarray(
        Wl.reshape(6, P, NCLS).transpose(1, 0, 2)
    ).astype(np.float16)  # [128, 6, 7]
    return {
        "w1": chunk_rhs(w1cat, 4, np.float16),
        "w2": chunk_rhs(w2cat, 2, np.float16),
        "w3": chunk_rhs(w3cat, 2, np.float16),
        "wl": wl,
        "b1": np.tile(b1[None, :], (P, 1)).astype(np.float32),
        "b2": np.tile(b2[None, :], (P, 1)).astype(np.float32),
        "b3": np.tile(b3[None, :], (P, 1)).astype(np.float32),
        "bl": np.tile(bl[None, :], (P, 1)).astype(np.float32),
    }


# --------------------------------------------------------------------------
# device kernel
# --------------------------------------------------------------------------

def build(struct):
    cb = struct["cb"]
    cb4 = struct["cb4"]
    go4 = struct["go4"]
    kc = struct["kc"]
    kv = struct["kv"]
    voff = struct["voff"]
    vct = struct["vct"]
    gmax = struct["gmax"]
    ng4 = TPC // G4
    cb4max = [int(cb4[:, w].max()) for w in range(4)]

    nc = bacc.Bacc(
        "TRN2", target_bir_lowering=False, debug=False, num_devices=NC,
        num_swdge_queues=4,
    )

    xTc = nc.dram_tensor("xTc", [FPAD, NPC], f16, kind="ExternalInput")
    mdl_d = nc.dram_tensor("mdl", [P, vct], f16, kind="ExternalInput")
    mwn_d = nc.dram_tensor("mwn", [P, vct], f16, kind="ExternalInput")
    sdh_d = nc.dram_tensor("sdh", [P, TPC * 6, P], f16, kind="ExternalInput")
    gidx_d = nc.dram_tensor("gidx", [ng4, P, gmax], i16, kind="ExternalInput")
    w1_d = nc.dram_tensor("w1", [P, 4, 512], f16, kind="ExternalInput")
    w2_d = nc.dram_tensor("w2", [P, 2, 512], f16, kind="ExternalInput")
    w3_d = nc.dram_tensor("w3", [P, 2, 512], f16, kind="ExternalInput")
    wl_d = nc.dram_tensor("wl", [P, 6, NCLS], f16, kind="ExternalInput")
    b1_d = nc.dram_tensor("b1", [P, HID], f32, kind="ExternalInput")
    b2_d = nc.dram_tensor("b2", [P, HID], f32, kind="ExternalInput")
    b3_d = nc.dram_tensor("b3", [P, HID], f32, kind="ExternalInput")
    bl_d = nc.dram_tensor("bl", [P, NCLS], f32, kind="ExternalInput")
    out_d = nc.dram_tensor("out", [NPC, NCLS], f32, kind="ExternalOutput")

    with tile.TileContext(nc) as tc:
        with (
            tc.tile_pool(name="dram", bufs=1, space="DRAM") as dram,
            tc.tile_pool(name="const", bufs=1) as cpool,
            tc.tile_pool(name="lx", bufs=4) as lxpool,
            tc.tile_pool(name="stage", bufs=4) as stpool,
            tc.tile_pool(name="msga", bufs=LA + 1) as msgapool,
            tc.tile_pool(name="msgb", bufs=4) as msgbpool,
            tc.tile_pool(name="sbuild", bufs=2) as sbpool,
            tc.tile_pool(name="psa", bufs=2, space="PSUM") as psa,
            tc.tile_pool(name="psagg", bufs=2, space="PSUM") as psagg,
            tc.tile_pool(name="pstr", bufs=2, space="PSUM") as pstr,
            tc.tile_pool(name="pscls", bufs=2, space="PSUM") as pscls,
        ):
            # ---- DRAM intermediates
            p_loc = [
                [dram.tile([HROWS, HID], f16, name=f"p{i}loc{h}")
                 for h in range(2)]
                for i in range(3)
            ]
            p_full = [
                [dram.tile([TAB, HID], f16, addr_space="Shared",
                           name=f"p{i}full{h}") for h in range(2)]
                for i in range(3)
            ]
            r_dram = [dram.tile([NPC, HID], f16, name=f"r{i}d") for i in range(3)]

            # ---- constants
            w1_sb = cpool.tile([P, 4, 512], f16)
            nc.sync.dma_start(w1_sb[:], w1_d[:])
            w2_sb = cpool.tile([P, 2, 512], f16)
            nc.sync.dma_start(w2_sb[:], w2_d[:])
            w3_sb = cpool.tile([P, 2, 512], f16)
            nc.sync.dma_start(w3_sb[:], w3_d[:])
            wl_sb = cpool.tile([P, 6, NCLS], f16)
            nc.sync.dma_start(wl_sb[:], wl_d[:])
            b_sb = []
            for name, t in (("b1", b1_d), ("b2", b2_d), ("b3", b3_d)):
                bt = cpool.tile([P, HID], f32, name=name + "sb")
                nc.sync.dma_start(bt[:], t[:])
                b_sb.append(bt)
            bl_sb = cpool.tile([P, NCLS], f32)
            nc.sync.dma_start(bl_sb[:], bl_d[:])

            gidx_sb = cpool.tile([P, ng4, gmax], i16)
            nc.sync.dma_start(
                gidx_sb[:],
                gidx_d[:].rearrange("g p c -> p g c"),
            )

            ident = cpool.tile([P, P], f16)
            make_identity(nc, ident[:])

            mdl_sb = cpool.tile([P, vct], f16)
            nc.sync.dma_start(mdl_sb[:], mdl_d[:])
            mwn_sb = cpool.tile([P, vct], f16)
            nc.sync.dma_start(mwn_sb[:], mwn_d[:])
            iota_i = cpool.tile([P, P], i32)
            nc.gpsimd.iota(iota_i[:], pattern=[[1, P]], base=0,
                           channel_multiplier=0)
            iota_f = cpool.tile([P, P], f16)
            nc.vector.tensor_copy(iota_f[:], iota_i[:])

            lgall = cpool.tile([P, TPC, NCLS], f32)
            sm_all = cpool.tile([P, TPC], f32)

            # ---- layer-1 phase A (by pairs of tiles): r = x@Wtop+b, p = x@Wbot
            def pair_rows(buf, pr):
                return buf[pr * 2 * P:(pr + 1) * 2 * P, :].rearrange(
                    "(t p) c -> p t c", p=P
                )

            def phase_a_pair(pr):
                half, lpr = pr // (HT // 2), pr % (HT // 2)
                lx = lxpool.tile([P, 4, 2 * P], f16, name="lx", tag="lx")
                nc.sync.dma_start(
                    lx[:],
                    xTc[:, pr * 2 * P:(pr + 1) * 2 * P].rearrange(
                        "(k p) c -> p k c", p=P
                    ),
                )
                rst = stpool.tile([P, 2, HID], f16, name="rst", tag="rst")
                pst = stpool.tile([P, 2, HID], f16, name="pst", tag="pst")
                for gt in range(2):
                    ps = psa.tile([P, 512], f32, name="psA", tag="psA")
                    for k in range(4):
                        nc.tensor.matmul(
                            out=ps[:], lhsT=lx[:, k, gt * P:(gt + 1) * P],
                            rhs=w1_sb[:, k, :],
                            start=(k == 0), stop=(k == 3),
                        )
                    nc.vector.tensor_tensor(
                        out=rst[:, gt, :], in0=ps[:, :HID], in1=b_sb[0][:],
                        op=mybir.AluOpType.add,
                    )
                    nc.vector.tensor_copy(pst[:, gt, :], ps[:, HID:])
                nc.sync.dma_start(pair_rows(r_dram[0], pr), rst[:])
                nc.sync.dma_start(pair_rows(p_loc[0][half], lpr), pst[:])

            def fire_ag(layer, half):
                nc.gpsimd.collective_compute(
                    "AllGather",
                    mybir.AluOpType.bypass,
                    replica_groups=[list(range(NC))],
                    ins=[p_loc[layer][half].opt()],
                    outs=[p_full[layer][half].opt()],
                )

            # ---- phase B by groups of G4 tiles
            def issue_gathers(layer, g, ws):
                ms = {}
                for w in ws:
                    pool = msgapool if w < 2 else msgbpool
                    m = pool.tile(
                        [P, cb4max[w], HID], f16, name=f"m{w}", tag=f"m{w}"
                    )
                    cols = int(cb4[g, w])
                    ni = cols * P
                    nc.gpsimd.dma_gather(
                        out_ap=m[:, :cols, :],
                        in_ap=p_full[layer][w // 2][
                            (W1BASE if w % 2 else 0):, :
                        ],
                        idxs_ap=gidx_sb[:, g, int(go4[g, w]):int(go4[g, w + 1])],
                        num_idxs=ni,
                        num_idxs_reg=ni,
                        elem_size=HID,
                        single_packet=(ni <= 1024),
                        queue_num=w,
                    )
                    ms[w] = m
                return ms

            def phase_b_group(layer, g, msgs):
                last = layer == 2
                t0 = g * G4
                kvg = int(voff[t0 + G4] - voff[t0])
                o0 = int(voff[t0])
                s_v = sbpool.tile([P, kvg, P], f16, name="sv", tag="sv")
                nc.vector.tensor_tensor(
                    out=s_v[:],
                    in0=mdl_sb[:, o0:o0 + kvg].unsqueeze(2).broadcast_to(
                        [P, kvg, P]
                    ),
                    in1=iota_f[:].unsqueeze(1).broadcast_to([P, kvg, P]),
                    op=mybir.AluOpType.is_equal,
                )
                nc.vector.tensor_tensor(
                    out=s_v[:],
                    in0=s_v[:],
                    in1=mwn_sb[:, o0:o0 + kvg].unsqueeze(2).broadcast_to(
                        [P, kvg, P]
                    ),
                    op=mybir.AluOpType.mult,
                )
                s_h = sbpool.tile([P, G4 * 6, P], f16, name="sh", tag="sh")
                nc.sync.dma_start(
                    s_h[:], sdh_d[:, t0 * 6:(t0 + G4) * 6, :]
                )
                rst = stpool.tile([P, G4, HID], f16, name="rl", tag="rl")
                nc.sync.dma_start(
                    rst[:],
                    r_dram[layer][t0 * P:(t0 + G4) * P, :].rearrange(
                        "(t p) c -> p t c", p=P
                    ),
                )
                aggs = []
                for ti in range(G4):
                    tl = t0 + ti
                    agg = psagg.tile([P, HID], f32, name="agg", tag="agg")
                    nchunks = int(kc[tl])
                    cv = int(voff[tl]) - o0
                    done = 0
                    for w in range(4):
                        cbo = int(cb[t0:tl, w].sum())
                        for j in range(int(cb[tl, w])):
                            if w % 2 == 0:
                                lhs = s_h[:, ti * 6 + (0 if w == 0 else 3) + j, :]
                            else:
                                lhs = s_v[:, cv, :]
                                cv += 1
                            nc.tensor.matmul(
                                out=agg[:],
                                lhsT=lhs,
                                rhs=msgs[w][:, cbo + j, :],
                                start=(done == 0),
                                stop=(done == nchunks - 1),
                            )
                            done += 1
                    aggs.append(agg)
                for ti in range(G4):
                    tl = t0 + ti
                    agg = aggs[ti]
                    xsum = stpool.tile([P, HID], f16, name="xsum", tag="xsum")
                    nc.vector.tensor_tensor(
                        out=xsum[:], in0=agg[:], in1=rst[:, ti, :],
                        op=mybir.AluOpType.add,
                    )
                    xn = stpool.tile([P, HID], f16, name="xn", tag="xn")
                    nc.scalar.activation(
                        xn[:], xsum[:], mybir.ActivationFunctionType.Relu
                    )
                    xts = []
                    for h in range(2):
                        tp = pstr.tile([P, P], f16, name="tp", tag="tp")
                        nc.tensor.transpose(
                            out=tp[:], in_=xn[:, h * P:(h + 1) * P],
                            identity=ident[:],
                        )
                        xt = stpool.tile([P, P], f16, name=f"xt{h}",
                                         tag=f"xt{h}")
                        nc.vector.tensor_copy(xt[:], tp[:])
                        xts.append(xt)
                    # fused next-layer phase A: r/p for layer+1 from x in SBUF
                    if not last:
                        half, lt = tl // HT, tl % HT
                        wsb = w2_sb if layer == 0 else w3_sb
                        ps = psa.tile([P, 512], f32, name="psA", tag="psA")
                        for k in range(2):
                            nc.tensor.matmul(
                                out=ps[:], lhsT=xts[k][:], rhs=wsb[:, k, :],
                                start=(k == 0), stop=(k == 1),
                            )
                        rs2 = stpool.tile([P, HID], f16, name="rs2", tag="rs2")
                        nc.vector.tensor_tensor(
                            out=rs2[:], in0=ps[:, :HID], in1=b_sb[layer + 1][:],
                            op=mybir.AluOpType.add,
                        )
                        ps2 = stpool.tile([P, HID], f16, name="ps2", tag="ps2")
                        nc.vector.tensor_copy(ps2[:], ps[:, HID:])
                        nc.sync.dma_start(
                            r_dram[layer + 1][tl * P:(tl + 1) * P, :].rearrange(
                                "(o p) c -> p (o c)", p=P
                            ),
                            rs2[:],
                        )
                        nc.sync.dma_start(
                            p_loc[layer + 1][half][
                                lt * P:(lt + 1) * P, :
                            ].rearrange("(o p) c -> p (o c)", p=P),
                            ps2[:],
                        )
                    # incremental classifier: lgall += x_l^T @ Wl[2l:2l+2]
                    cls = pscls.tile([P, NCLS], f32, name="cls", tag="cls")
                    for kk in range(2):
                        nc.tensor.matmul(
                            out=cls[:], lhsT=xts[kk][:],
                            rhs=wl_sb[:, 2 * layer + kk, :],
                            start=(kk == 0), stop=(kk == 1),
                        )
                    if layer == 0:
                        nc.vector.tensor_copy(lgall[:, tl, :], cls[:])
                    else:
                        nc.vector.tensor_tensor(
                            out=lgall[:, tl, :], in0=lgall[:, tl, :],
                            in1=cls[:], op=mybir.AluOpType.add,
                        )
                    if last:
                        nc.vector.tensor_tensor(
                            out=lgall[:, tl, :], in0=lgall[:, tl, :],
                            in1=bl_sb[:], op=mybir.AluOpType.add,
                        )

            # batched shifted-exp over a block of tiles (layer 3)
            def softmax_block(t0, nt):
                mx = stpool.tile([P, nt], f32, name="mx", tag="mx")
                nc.vector.tensor_reduce(
                    out=mx[:], in_=lgall[:, t0:t0 + nt, :],
                    axis=mybir.AxisListType.X, op=mybir.AluOpType.max,
                )
                nc.vector.tensor_tensor(
                    out=lgall[:, t0:t0 + nt, :], in0=lgall[:, t0:t0 + nt, :],
                    in1=mx[:].unsqueeze(2).broadcast_to([P, nt, NCLS]),
                    op=mybir.AluOpType.subtract,
                )
                ex = stpool.tile([P, nt, NCLS], f32, name="ex", tag="ex")
                nc.scalar.activation(
                    ex[:], lgall[:, t0:t0 + nt, :],
                    mybir.ActivationFunctionType.Exp,
                )
                nc.vector.tensor_reduce(
                    out=sm_all[:, t0:t0 + nt], in_=ex[:],
                    axis=mybir.AxisListType.X, op=mybir.AluOpType.add,
                )

            # ---- drive: layer-1 phase A, then 3 fused phase-B passes
            for pr in range(HT):
                phase_a_pair(pr)
                if pr == HT // 2 - 1:
                    fire_ag(0, 0)
            fire_ag(0, 1)
            for layer in range(3):
                # lookahead window-0/1 gathers overlap AllGather b's flight
                amsgs = [issue_gathers(layer, g, (0, 1)) for g in range(LA)]
                for g in range(ng4):
                    msgs = issue_gathers(layer, g, (2, 3))
                    if g + LA < ng4:
                        amsgs.append(issue_gathers(layer, g + LA, (0, 1)))
                    msgs.update(amsgs[g])
                    phase_b_group(layer, g, msgs)
                    if layer < 2:
                        if g == HT // G4 - 1:
                            fire_ag(layer + 1, 0)
                        elif g == ng4 - 1:
                            fire_ag(layer + 1, 1)
                    else:
                        softmax_block(g * G4, G4)

            lsm = cpool.tile([P, TPC], f32)
            nc.scalar.activation(
                lsm[:], sm_all[:], mybir.ActivationFunctionType.Ln
            )
            nc.vector.tensor_tensor(
                out=lgall[:], in0=lgall[:],
                in1=lsm[:].unsqueeze(2).broadcast_to([P, TPC, NCLS]),
                op=mybir.AluOpType.subtract,
            )
            nc.sync.dma_start(
                out_d[:].rearrange("(t p) j -> p t j", p=P), lgall[:]
            )

    nc.compile()
    return nc


# --------------------------------------------------------------------------
# entry point
# --------------------------------------------------------------------------

def kernel(x, edge_index, edge_weight, W1, b1, W2, b2, W3, b3, Wl, bl):
    x = np.asarray(x, dtype=np.float32)
    edge_index = np.asarray(edge_index)
    edge_weight = np.asarray(edge_weight, dtype=np.float32)

    pp = prep(x, edge_index, edge_weight)
    wts = pack_weights(
        np.asarray(W1, np.float32), np.asarray(b1, np.float32),
        np.asarray(W2, np.float32), np.asarray(b2, np.float32),
        np.asarray(W3, np.float32), np.asarray(b3, np.float32),
        np.asarray(Wl, np.float32), np.asarray(bl, np.float32),
    )

    key = (pp["vct"], pp["gmax"], tuple(pp["cb"].reshape(-1).tolist()))
    if key not in _compile_cache:
        _compile_cache[key] = build(pp)
    nc = _compile_cache[key]

    in_maps = []
    for c in range(NC):
        in_maps.append({
            "xTc": np.ascontiguousarray(pp["xT"][:, c * NPC:(c + 1) * NPC]),
            "mdl": np.ascontiguousarray(pp["mdl"][c]),
            "mwn": np.ascontiguousarray(pp["mwn"][c]),
            "sdh": np.ascontiguousarray(pp["sdh"][c]),
            "gidx": np.ascontiguousarray(pp["gidx"][c]),
            **wts,
        })

    res = run_bass_kernel_spmd(nc, in_maps, list(range(NC)))
    out_full = np.concatenate([res.results[c]["out"] for c in range(NC)], axis=0)
    return out_full[pp["newpos"][:N]].astype(np.float32)


if __name__ == "__main__":
    import time

    rng = np.random.default_rng(0)
    E = 899756
    ei = rng.integers(0, N, (2, E)).astype(np.int32)
    ew = rng.random(E, dtype=np.float32)
    x = rng.standard_normal((N, IN_F), dtype=np.float32)
    t0 = time.time()
    pp = prep(x, ei, ew)
    print("prep", time.time() - t0, "sct =", pp["sct"], "gmax =", pp["gmax"])


# revision 72
# speedup vs baseline: 1.0241x; 1.0007x over previous
"""3-layer GraphSAGE + classifier + log_softmax on 8 Trainium2 NeuronCores.

Self-contained: host-side sharding/packing + Bass/Tile device kernel.

Strategy
--------
concat([x, agg]) @ W  ==  x @ W_top + Ahat @ (x @ W_bot)   (linearity)
so aggregation happens in the 256-dim projected space.

- Nodes are permuted into 704 tiles of 128 (in-degree balanced), 88 tiles/core.
- Per layer: phase A computes r = x@W_top + b and p = x@W_bot per owned tile.
  p is written fp16 in two halves; each half is AllGathered into its own
  table (45056 rows) as soon as the half is computed, hiding collective
  latency under the remaining phase-A work.
- Phase B: per dst tile, gather p[src] rows for its in-edges via dma_gather
  (int16 indices; 4 overlapping windows, 2 per table). Window 0/2 carry a
  fixed 384 edges per tile (zero padding); windows 1/3 carry the remainder
  with trailing -1 indices that the gather ucode truncates per core.
  Gathers round-robin the 4 SWDGE queues so all 4 Q7 core pairs generate
  descriptors concurrently. The weighted one-hot selection matrices
  S[e, d] = wn_e * (dst_local_e == d) are precomputed on the host and
  DMAed; agg = sum_c S_c.T @ msg_c accumulates on the PE.
  x_next = relu(agg + r).
- x_next is transposed on the PE (2x 128x128) to feed the next layer's
  stationary operand; the classifier (768->7) runs per tile in layer-3
  phase B; the log_softmax runs batched (max/exp/sum per 4-tile block,
  single ln at the end).
"""

import numpy as np

import concourse.bass as bass
import concourse.mybir as mybir
import concourse.tile as tile
from concourse import bacc
from concourse.bass_utils import run_bass_kernel_spmd
from concourse.masks import make_identity

# problem constants
N = 89250
IN_F = 500
HID = 256
NCLS = 7
FPAD = 512  # padded input feature dim

NC = 8  # cores
P = 128
NT = 704  # node tiles
TPC = NT // NC  # 88 tiles per core
NPAD = NT * P  # 90112
NPC = TPC * P  # 11264 nodes per core
HT = TPC // 2  # 44 tiles per half
HROWS = HT * P  # 5632 rows per half per core
TAB = NC * HROWS  # 45056 rows per AllGathered table

WCAP = 32768  # int16 index reach
W1BASE = TAB - WCAP  # 12288: base row of windows 1/3 within their table
SPLIT0 = 384  # fixed edges per tile routed to window 0 (and window 2)
G4 = 4  # tiles per gather group
LA = 3  # lookahead groups for window-0/1 gathers (hide AllGather b)

f32 = mybir.dt.float32
f16 = mybir.dt.float16
i16 = mybir.dt.int16
i32 = mybir.dt.int32

_compile_cache = {}


# --------------------------------------------------------------------------
# host-side prep
# --------------------------------------------------------------------------

def _assign_tiles(in_deg):
    """LPT: assign node ids (0..NPAD) to (tile, slot), balancing in-edges."""
    import heapq

    order = np.argsort(-in_deg, kind="stable")
    heap = [(0, t) for t in range(NT)]
    heapq.heapify(heap)
    counts = np.zeros(NT, np.int32)
    newpos = np.empty(NPAD, np.int64)
    for v in order:
        load, t = heapq.heappop(heap)
        newpos[v] = t * P + counts[t]
        counts[t] += 1
        if counts[t] < P:
            heapq.heappush(heap, (load + int(in_deg[v]), t))
    return newpos


def _ru16(x):
    return (int(x) + 15) // 16 * 16


def prep(x, edge_index, edge_weight):
    src = edge_index[0].astype(np.int64)
    dst = edge_index[1].astype(np.int64)
    ew = edge_weight.astype(np.float32)

    cnt = np.bincount(dst, minlength=N).astype(np.float32)
    wn = ew / np.maximum(cnt[dst], 1.0)

    in_deg = np.zeros(NPAD, np.int64)
    in_deg[:N] = np.bincount(dst, minlength=N)
    newpos = _assign_tiles(in_deg)

    s2 = newpos[src]
    d2 = newpos[dst]
    # table row of each source: owner core c, local row jl; half A = first
    # 44 tiles of the core, half B = rest.  trow = c*HROWS + (jl mod HROWS)
    c_own = s2 // NPC
    jl = s2 % NPC
    is_b = jl >= HROWS
    trow = c_own * HROWS + (jl - HROWS * is_b)

    tile_of = d2 // P
    dl = (d2 % P).astype(np.int64)

    order = np.argsort(tile_of, kind="stable")
    trow_o, isb_o, dl_o, wn_o = trow[order], is_b[order], dl[order], wn[order]
    tile_o = tile_of[order]
    starts = np.searchsorted(tile_o, np.arange(NT + 1))

    # per (tile, window): sorted index lists + (dl, wn) in slot order
    # windows: 0 = A[0:32768), 1 = A[12288:45056), 2/3 same for B
    tw_idx = [[None] * 4 for _ in range(NT)]
    tw_dl = [[None] * 4 for _ in range(NT)]
    tw_wn = [[None] * 4 for _ in range(NT)]
    for t in range(NT):
        lo, hi = starts[t], starts[t + 1]
        tr, ib = trow_o[lo:hi], isb_o[lo:hi]
        dd, ww = dl_o[lo:hi], wn_o[lo:hi]
        for half in range(2):
            sel = np.nonzero(ib == half)[0]
            o = sel[np.argsort(tr[sel], kind="stable")]
            n = len(o)
            assert n >= SPLIT0, f"tile {t} half {half}: only {n} edges"
            assert tr[o[SPLIT0 - 1]] < WCAP, f"tile {t}: w0 split infeasible"
            assert tr[o[SPLIT0]] >= W1BASE, f"tile {t}: w1 split infeasible"
            w0, w1 = 2 * half, 2 * half + 1
            tw_idx[t][w0] = tr[o[:SPLIT0]].astype(np.int16)
            tw_idx[t][w1] = (tr[o[SPLIT0:]] - W1BASE).astype(np.int16)
            tw_dl[t][w0], tw_dl[t][w1] = dd[o[:SPLIT0]], dd[o[SPLIT0:]]
            tw_wn[t][w0], tw_wn[t][w1] = ww[o[:SPLIT0]], ww[o[SPLIT0:]]

    # per tile-slot chunk budgets (uniform across cores for SPMD); tiles are
    # chunk-aligned inside group gathers, so pad each (tile, window) to a
    # multiple of 128 with repeats of the last index (row-buffer-hit reads)
    cb = np.zeros((TPC, 4), np.int64)
    for tl in range(TPC):
        for w in range(4):
            mx = max(len(tw_idx[c * TPC + tl][w]) for c in range(NC))
            cb[tl, w] = (mx + P - 1) // P
    kc = cb.sum(axis=1)  # chunks per tile
    soff = np.zeros(TPC + 1, np.int64)
    soff[1:] = np.cumsum(kc)
    sct = int(soff[-1])

    # group-of-G4 gather packing: per (group, window) one gather whose index
    # list is the concat of the group's tiles (each padded to cb*128)
    ng4 = TPC // G4
    cb4 = np.zeros((ng4, 4), np.int64)  # chunks per (group, window)
    for g in range(ng4):
        cb4[g] = cb[g * G4:(g + 1) * G4].sum(axis=0)
    go4 = np.zeros((ng4, 5), np.int64)  # gidx column offsets (int16 cols)
    for g in range(ng4):
        go4[g, 1:] = np.cumsum(cb4[g] * 8)
    gmax = int(go4[:, 4].max())

    # S chunk numbering: per tile, w1 chunks then w3 chunks (DVE-built);
    # w0/w2 chunks (3 each, fixed) are host-precomputed dense and DMAed
    kv = (cb[:, 1] + cb[:, 3]).astype(np.int64)  # DVE-built chunks per tile
    voff = np.zeros(TPC + 1, np.int64)
    voff[1:] = np.cumsum(kv)
    vct = int(voff[-1])

    gidx = np.zeros((NC, ng4, P, gmax), np.int16)
    mdl = np.zeros((NC, P, vct), np.float16)
    mwn = np.zeros((NC, P, vct), np.float16)
    sdh = np.zeros((NC, P, TPC * 6, P), np.float16)
    for t in range(NT):
        c, tl = t // TPC, t % TPC
        g, ti = tl // G4, tl % G4
        cv0 = 0
        for w in range(4):
            idx = tw_idx[t][w]
            n = len(idx)
            b = int(cb[tl, w]) * P
            arr = np.full(b, idx[-1], np.int16)
            arr[:n] = idx
            wrapped = arr.reshape(-1, 16).T  # [16, b/16]
            coff = int(go4[g, w]) + int(cb[g * G4:tl, w].sum()) * 8
            gidx[c, g, :, coff:coff + b // 16] = np.tile(wrapped, (8, 1))
            sl = np.arange(n)
            if w % 2 == 0:  # dense host S: w0 -> chunks 0-2, w2 -> 3-5
                h0 = tl * 6 + (0 if w == 0 else 3)
                sdh[c, sl % P, h0 + sl // P,
                    tw_dl[t][w].astype(np.int64)] = tw_wn[t][w]
            else:
                mdl[c, sl % P, voff[tl] + cv0 + sl // P] = tw_dl[t][w]
                mwn[c, sl % P, voff[tl] + cv0 + sl // P] = tw_wn[t][w]
                cv0 += int(cb[tl, w])

    # transposed, padded, permuted node features
    xT = np.zeros((FPAD, NPAD), np.float16)
    xT[:IN_F, newpos[:N]] = x.T

    return {
        "newpos": newpos,
        "cb": cb,
        "cb4": cb4,
        "go4": go4,
        "kc": kc,
        "kv": kv,
        "voff": voff,
        "vct": vct,
        "gmax": gmax,
        "xT": xT,
        "mdl": mdl,
        "mwn": mwn,
        "sdh": sdh,
        "gidx": gidx,
    }


def pack_weights(W1, b1, W2, b2, W3, b3, Wl, bl):
    def chunk_rhs(W, kchunks, dtype):
        # [F, 512] -> [128, kchunks, 512]
        F = W.shape[0]
        Wp = np.zeros((kchunks * P, 512), np.float32)
        Wp[:F] = W
        return np.ascontiguousarray(
            Wp.reshape(kchunks, P, 512).transpose(1, 0, 2)
        ).astype(dtype)

    w1cat = np.concatenate([W1[:IN_F], W1[IN_F:]], axis=1)  # [500, 512]
    w2cat = np.concatenate([W2[:HID], W2[HID:]], axis=1)  # [256, 512]
    w3cat = np.concatenate([W3[:HID], W3[HID:]], axis=1)
    wl = np.ascontiguousarray(
        Wl.reshape(6, P, NCLS).transpose(1, 0, 2)
    ).astype(np.float16)  # [128, 6, 7]
    return {
        "w1": chunk_rhs(w1cat, 4, np.float16),
        "w2": chunk_rhs(w2cat, 2, np.float16),
        "w3": chunk_rhs(w3cat, 2, np.float16),
        "wl": wl,
        "b1": np.tile(b1[None, :], (P, 1)).astype(np.float32),
        "b2": np.tile(b2[None, :], (P, 1)).astype(np.float32),
        "b3": np.tile(b3[None, :], (P, 1)).astype(np.float32),
        "bl": np.tile(bl[None, :], (P, 1)).astype(np.float32),
    }


# --------------------------------------------------------------------------
# device kernel
# --------------------------------------------------------------------------

def build(struct):
    cb = struct["cb"]
    cb4 = struct["cb4"]
    go4 = struct["go4"]
    kc = struct["kc"]
    kv = struct["kv"]
    voff = struct["voff"]
    vct = struct["vct"]
    gmax = struct["gmax"]
    ng4 = TPC // G4
    cb4max = [int(cb4[:, w].max()) for w in range(4)]

    nc = bacc.Bacc(
        "TRN2", target_bir_lowering=False, debug=False, num_devices=NC,
        num_swdge_queues=4,
    )

    xTc = nc.dram_tensor("xTc", [FPAD, NPC], f16, kind="ExternalInput")
    mdl_d = nc.dram_tensor("mdl", [P, vct], f16, kind="ExternalInput")
    mwn_d = nc.dram_tensor("mwn", [P, vct], f16, kind="ExternalInput")
    sdh_d = nc.dram_tensor("sdh", [P, TPC * 6, P], f16, kind="ExternalInput")
    gidx_d = nc.dram_tensor("gidx", [ng4, P, gmax], i16, kind="ExternalInput")
    w1_d = nc.dram_tensor("w1", [P, 4, 512], f16, kind="ExternalInput")
    w2_d = nc.dram_tensor("w2", [P, 2, 512], f16, kind="ExternalInput")
    w3_d = nc.dram_tensor("w3", [P, 2, 512], f16, kind="ExternalInput")
    wl_d = nc.dram_tensor("wl", [P, 6, NCLS], f16, kind="ExternalInput")
    b1_d = nc.dram_tensor("b1", [P, HID], f32, kind="ExternalInput")
    b2_d = nc.dram_tensor("b2", [P, HID], f32, kind="ExternalInput")
    b3_d = nc.dram_tensor("b3", [P, HID], f32, kind="ExternalInput")
    bl_d = nc.dram_tensor("bl", [P, NCLS], f32, kind="ExternalInput")
    out_d = nc.dram_tensor("out", [NPC, NCLS], f32, kind="ExternalOutput")

    with tile.TileContext(nc) as tc:
        with (
            tc.tile_pool(name="dram", bufs=1, space="DRAM") as dram,
            tc.tile_pool(name="const", bufs=1) as cpool,
            tc.tile_pool(name="lx", bufs=4) as lxpool,
            tc.tile_pool(name="stage", bufs=4) as stpool,
            tc.tile_pool(name="msga", bufs=LA + 1) as msgapool,
            tc.tile_pool(name="msgb", bufs=4) as msgbpool,
            tc.tile_pool(name="sbuild", bufs=2) as sbpool,
            tc.tile_pool(name="psa", bufs=2, space="PSUM") as psa,
            tc.tile_pool(name="psagg", bufs=2, space="PSUM") as psagg,
            tc.tile_pool(name="pstr", bufs=2, space="PSUM") as pstr,
            tc.tile_pool(name="pscls", bufs=2, space="PSUM") as pscls,
        ):
            # ---- DRAM intermediates
            p_loc = [
                [dram.tile([HROWS, HID], f16, name=f"p{i}loc{h}")
                 for h in range(2)]
                for i in range(3)
            ]
            p_full = [
                [dram.tile([TAB, HID], f16, addr_space="Shared",
                           name=f"p{i}full{h}") for h in range(2)]
                for i in range(3)
            ]
            r_dram = [dram.tile([NPC, HID], f16, name=f"r{i}d") for i in range(3)]

            # ---- constants
            w1_sb = cpool.tile([P, 4, 512], f16)
            nc.sync.dma_start(w1_sb[:], w1_d[:])
            w2_sb = cpool.tile([P, 2, 512], f16)
            nc.sync.dma_start(w2_sb[:], w2_d[:])
            w3_sb = cpool.tile([P, 2, 512], f16)
            nc.sync.dma_start(w3_sb[:], w3_d[:])
            wl_sb = cpool.tile([P, 6, NCLS], f16)
            nc.sync.dma_start(wl_sb[:], wl_d[:])
            b_sb = []
            for name, t in (("b1", b1_d), ("b2", b2_d), ("b3", b3_d)):
                bt = cpool.tile([P, HID], f32, name=name + "sb")
                nc.sync.dma_start(bt[:], t[:])
                b_sb.append(bt)
            bl_sb = cpool.tile([P, NCLS], f32)
            nc.sync.dma_start(bl_sb[:], bl_d[:])

            gidx_sb = cpool.tile([P, ng4, gmax], i16)
            nc.sync.dma_start(
                gidx_sb[:],
                gidx_d[:].rearrange("g p c -> p g c"),
            )

            ident = cpool.tile([P, P], f16)
            make_identity(nc, ident[:])

            mdl_sb = cpool.tile([P, vct], f16)
            nc.sync.dma_start(mdl_sb[:], mdl_d[:])
            mwn_sb = cpool.tile([P, vct], f16)
            nc.sync.dma_start(mwn_sb[:], mwn_d[:])
            iota_i = cpool.tile([P, P], i32)
            nc.gpsimd.iota(iota_i[:], pattern=[[1, P]], base=0,
                           channel_multiplier=0)
            iota_f = cpool.tile([P, P], f16)
            nc.vector.tensor_copy(iota_f[:], iota_i[:])

            lgall = cpool.tile([P, TPC, NCLS], f32)
            sm_all = cpool.tile([P, TPC], f32)

            # ---- layer-1 phase A (by pairs of tiles): r = x@Wtop+b, p = x@Wbot
            def pair_rows(buf, pr):
                return buf[pr * 2 * P:(pr + 1) * 2 * P, :].rearrange(
                    "(t p) c -> p t c", p=P
                )

            def phase_a_p(pr):
                """p = x @ W1_bot only — unblocks the AllGathers early."""
                half, lpr = pr // (HT // 2), pr % (HT // 2)
                lx = lxpool.tile([P, 4, 2 * P], f16, name="lx", tag="lx")
                nc.sync.dma_start(
                    lx[:],
                    xTc[:, pr * 2 * P:(pr + 1) * 2 * P].rearrange(
                        "(k p) c -> p k c", p=P
                    ),
                )
                pst = stpool.tile([P, 2, HID], f16, name="pst", tag="pst")
                for gt in range(2):
                    ps = psa.tile([P, HID], f32, name="psP", tag="psA")
                    for k in range(4):
                        nc.tensor.matmul(
                            out=ps[:], lhsT=lx[:, k, gt * P:(gt + 1) * P],
                            rhs=w1_sb[:, k, HID:],
                            start=(k == 0), stop=(k == 3),
                        )
                    nc.vector.tensor_copy(pst[:, gt, :], ps[:])
                nc.sync.dma_start(pair_rows(p_loc[0][half], lpr), pst[:])

            def phase_a_r(pr):
                """r = x @ W1_top + b — overlaps the AllGather flight."""
                lx = lxpool.tile([P, 4, 2 * P], f16, name="lx", tag="lx")
                nc.sync.dma_start(
                    lx[:],
                    xTc[:, pr * 2 * P:(pr + 1) * 2 * P].rearrange(
                        "(k p) c -> p k c", p=P
                    ),
                )
                rst = stpool.tile([P, 2, HID], f16, name="rst", tag="rst")
                for gt in range(2):
                    ps = psa.tile([P, HID], f32, name="psP", tag="psA")
                    for k in range(4):
                        nc.tensor.matmul(
                            out=ps[:], lhsT=lx[:, k, gt * P:(gt + 1) * P],
                            rhs=w1_sb[:, k, :HID],
                            start=(k == 0), stop=(k == 3),
                        )
                    nc.vector.tensor_tensor(
                        out=rst[:, gt, :], in0=ps[:], in1=b_sb[0][:],
                        op=mybir.AluOpType.add,
                    )
                nc.sync.dma_start(pair_rows(r_dram[0], pr), rst[:])

            def fire_ag(layer, half):
                nc.gpsimd.collective_compute(
                    "AllGather",
                    mybir.AluOpType.bypass,
                    replica_groups=[list(range(NC))],
                    ins=[p_loc[layer][half].opt()],
                    outs=[p_full[layer][half].opt()],
                )

            # ---- phase B by groups of G4 tiles
            def issue_gathers(layer, g, ws):
                ms = {}
                for w in ws:
                    pool = msgapool if w < 2 else msgbpool
                    m = pool.tile(
                        [P, cb4max[w], HID], f16, name=f"m{w}", tag=f"m{w}"
                    )
                    cols = int(cb4[g, w])
                    ni = cols * P
                    nc.gpsimd.dma_gather(
                        out_ap=m[:, :cols, :],
                        in_ap=p_full[layer][w // 2][
                            (W1BASE if w % 2 else 0):, :
                        ],
                        idxs_ap=gidx_sb[:, g, int(go4[g, w]):int(go4[g, w + 1])],
                        num_idxs=ni,
                        num_idxs_reg=ni,
                        elem_size=HID,
                        single_packet=(ni <= 1024),
                        queue_num=w,
                    )
                    ms[w] = m
                return ms

            def phase_b_group(layer, g, msgs):
                last = layer == 2
                t0 = g * G4
                kvg = int(voff[t0 + G4] - voff[t0])
                o0 = int(voff[t0])
                s_v = sbpool.tile([P, kvg, P], f16, name="sv", tag="sv")
                nc.vector.tensor_tensor(
                    out=s_v[:],
                    in0=mdl_sb[:, o0:o0 + kvg].unsqueeze(2).broadcast_to(
                        [P, kvg, P]
                    ),
                    in1=iota_f[:].unsqueeze(1).broadcast_to([P, kvg, P]),
                    op=mybir.AluOpType.is_equal,
                )
                nc.vector.tensor_tensor(
                    out=s_v[:],
                    in0=s_v[:],
                    in1=mwn_sb[:, o0:o0 + kvg].unsqueeze(2).broadcast_to(
                        [P, kvg, P]
                    ),
                    op=mybir.AluOpType.mult,
                )
                s_h = sbpool.tile([P, G4 * 6, P], f16, name="sh", tag="sh")
                nc.sync.dma_start(
                    s_h[:], sdh_d[:, t0 * 6:(t0 + G4) * 6, :]
                )
                rst = stpool.tile([P, G4, HID], f16, name="rl", tag="rl")
                nc.sync.dma_start(
                    rst[:],
                    r_dram[layer][t0 * P:(t0 + G4) * P, :].rearrange(
                        "(t p) c -> p t c", p=P
                    ),
                )
                aggs = []
                for ti in range(G4):
                    tl = t0 + ti
                    agg = psagg.tile([P, HID], f32, name="agg", tag="agg")
                    nchunks = int(kc[tl])
                    cv = int(voff[tl]) - o0
                    done = 0
                    for w in range(4):
                        cbo = int(cb[t0:tl, w].sum())
                        for j in range(int(cb[tl, w])):
                            if w % 2 == 0:
                                lhs = s_h[:, ti * 6 + (0 if w == 0 else 3) + j, :]
                            else:
                                lhs = s_v[:, cv, :]
                                cv += 1
                            nc.tensor.matmul(
                                out=agg[:],
                                lhsT=lhs,
                                rhs=msgs[w][:, cbo + j, :],
                                start=(done == 0),
                                stop=(done == nchunks - 1),
                            )
                            done += 1
                    aggs.append(agg)
                for ti in range(G4):
                    tl = t0 + ti
                    agg = aggs[ti]
                    xsum = stpool.tile([P, HID], f16, name="xsum", tag="xsum")
                    nc.vector.tensor_tensor(
                        out=xsum[:], in0=agg[:], in1=rst[:, ti, :],
                        op=mybir.AluOpType.add,
                    )
                    xn = stpool.tile([P, HID], f16, name="xn", tag="xn")
                    nc.scalar.activation(
                        xn[:], xsum[:], mybir.ActivationFunctionType.Relu
                    )
                    xts = []
                    for h in range(2):
                        tp = pstr.tile([P, P], f16, name="tp", tag="tp")
                        nc.tensor.transpose(
                            out=tp[:], in_=xn[:, h * P:(h + 1) * P],
                            identity=ident[:],
                        )
                        xt = stpool.tile([P, P], f16, name=f"xt{h}",
                                         tag=f"xt{h}")
                        nc.vector.tensor_copy(xt[:], tp[:])
                        xts.append(xt)
                    # fused next-layer phase A: r/p for layer+1 from x in SBUF
                    if not last:
                        half, lt = tl // HT, tl % HT
                        wsb = w2_sb if layer == 0 else w3_sb
                        ps = psa.tile([P, 512], f32, name="psA", tag="psA")
                        for k in range(2):
                            nc.tensor.matmul(
                                out=ps[:], lhsT=xts[k][:], rhs=wsb[:, k, :],
                                start=(k == 0), stop=(k == 1),
                            )
                        rs2 = stpool.tile([P, HID], f16, name="rs2", tag="rs2")
                        nc.vector.tensor_tensor(
                            out=rs2[:], in0=ps[:, :HID], in1=b_sb[layer + 1][:],
                            op=mybir.AluOpType.add,
                        )
                        ps2 = stpool.tile([P, HID], f16, name="ps2", tag="ps2")
                        nc.vector.tensor_copy(ps2[:], ps[:, HID:])
                        nc.sync.dma_start(
                            r_dram[layer + 1][tl * P:(tl + 1) * P, :].rearrange(
                                "(o p) c -> p (o c)", p=P
                            ),
                            rs2[:],
                        )
                        nc.sync.dma_start(
                            p_loc[layer + 1][half][
                                lt * P:(lt + 1) * P, :
                            ].rearrange("(o p) c -> p (o c)", p=P),
                            ps2[:],
                        )
                    # incremental classifier: lgall += x_l^T @ Wl[2l:2l+2]
                    cls = pscls.tile([P, NCLS], f32, name="cls", tag="cls")
                    for kk in range(2):
                        nc.tensor.matmul(
                            out=cls[:], lhsT=xts[kk][:],
                            rhs=wl_sb[:, 2 * layer + kk, :],
                            start=(kk == 0), stop=(kk == 1),
                        )
                    if layer == 0:
                        nc.vector.tensor_copy(lgall[:, tl, :], cls[:])
                    else:
                        nc.vector.tensor_tensor(
                            out=lgall[:, tl, :], in0=lgall[:, tl, :],
                            in1=cls[:], op=mybir.AluOpType.add,
                        )
                    if last:
                        nc.vector.tensor_tensor(
                            out=lgall[:, tl, :], in0=lgall[:, tl, :],
                            in1=bl_sb[:], op=mybir.AluOpType.add,
                        )

            # batched shifted-exp over a block of tiles (layer 3)
            def softmax_block(t0, nt):
                mx = stpool.tile([P, nt], f32, name="mx", tag="mx")
                nc.vector.tensor_reduce(
                    out=mx[:], in_=lgall[:, t0:t0 + nt, :],
                    axis=mybir.AxisListType.X, op=mybir.AluOpType.max,
                )
                nc.vector.tensor_tensor(
                    out=lgall[:, t0:t0 + nt, :], in0=lgall[:, t0:t0 + nt, :],
                    in1=mx[:].unsqueeze(2).broadcast_to([P, nt, NCLS]),
                    op=mybir.AluOpType.subtract,
                )
                ex = stpool.tile([P, nt, NCLS], f32, name="ex", tag="ex")
                nc.scalar.activation(
                    ex[:], lgall[:, t0:t0 + nt, :],
                    mybir.ActivationFunctionType.Exp,
                )
                nc.vector.tensor_reduce(
                    out=sm_all[:, t0:t0 + nt], in_=ex[:],
                    axis=mybir.AxisListType.X, op=mybir.AluOpType.add,
                )

            # ---- drive: layer-1 phase A (p first, AGs early, r under AG),
            # then 3 fused phase-B passes
            for pr in range(HT):
                phase_a_p(pr)
                if pr == HT // 2 - 1:
                    fire_ag(0, 0)
            fire_ag(0, 1)
            for pr in range(HT):
                phase_a_r(pr)
            for layer in range(3):
                # lookahead window-0/1 gathers overlap AllGather b's flight
                amsgs = [issue_gathers(layer, g, (0, 1)) for g in range(LA)]
                for g in range(ng4):
                    msgs = issue_gathers(layer, g, (2, 3))
                    if g + LA < ng4:
                        amsgs.append(issue_gathers(layer, g + LA, (0, 1)))
                    msgs.update(amsgs[g])
                    phase_b_group(layer, g, msgs)
                    if layer < 2:
                        if g == HT // G4 - 1:
                            fire_ag(layer + 1, 0)
                        elif g == ng4 - 1:
                            fire_ag(layer + 1, 1)
                    else:
                        softmax_block(g * G4, G4)

            lsm = cpool.tile([P, TPC], f32)
            nc.scalar.activation(
                lsm[:], sm_all[:], mybir.ActivationFunctionType.Ln
            )
            nc.vector.tensor_tensor(
                out=lgall[:], in0=lgall[:],
                in1=lsm[:].unsqueeze(2).broadcast_to([P, TPC, NCLS]),
                op=mybir.AluOpType.subtract,
            )
            nc.sync.dma_start(
                out_d[:].rearrange("(t p) j -> p t j", p=P), lgall[:]
            )

    nc.compile()
    return nc


# --------------------------------------------------------------------------
# entry point
# --------------------------------------------------------------------------

def kernel(x, edge_index, edge_weight, W1, b1, W2, b2, W3, b3, Wl, bl):
    x = np.asarray(x, dtype=np.float32)
    edge_index = np.asarray(edge_index)
    edge_weight = np.asarray(edge_weight, dtype=np.float32)

    pp = prep(x, edge_index, edge_weight)
    wts = pack_weights(
        np.asarray(W1, np.float32), np.asarray(b1, np.float32),
        np.asarray(W2, np.float32), np.asarray(b2, np.float32),
        np.asarray(W3, np.float32), np.asarray(b3, np.float32),
        np.asarray(Wl, np.float32), np.asarray(bl, np.float32),
    )

    key = (pp["vct"], pp["gmax"], tuple(pp["cb"].reshape(-1).tolist()))
    if key not in _compile_cache:
        _compile_cache[key] = build(pp)
    nc = _compile_cache[key]

    in_maps = []
    for c in range(NC):
        in_maps.append({
            "xTc": np.ascontiguousarray(pp["xT"][:, c * NPC:(c + 1) * NPC]),
            "mdl": np.ascontiguousarray(pp["mdl"][c]),
            "mwn": np.ascontiguousarray(pp["mwn"][c]),
            "sdh": np.ascontiguousarray(pp["sdh"][c]),
            "gidx": np.ascontiguousarray(pp["gidx"][c]),
            **wts,
        })

    res = run_bass_kernel_spmd(nc, in_maps, list(range(NC)))
    out_full = np.concatenate([res.results[c]["out"] for c in range(NC)], axis=0)
    return out_full[pp["newpos"][:N]].astype(np.float32)


if __name__ == "__main__":
    import time

    rng = np.random.default_rng(0)
    E = 899756
    ei = rng.integers(0, N, (2, E)).astype(np.int32)
    ew = rng.random(E, dtype=np.float32)
    x = rng.standard_normal((N, IN_F), dtype=np.float32)
    t0 = time.time()
    pp = prep(x, ei, ew)
    print("prep", time.time() - t0, "sct =", pp["sct"], "gmax =", pp["gmax"])
